# revision 1
# baseline (speedup 1.0000x reference)
"""Pairwise L2-distance kernel (retrieval_knn) for 8x Trainium2 NeuronCores.

Computes Z = beta - sqrt(max(||x||^2 + ||y||^2 - 2 X@Y, 0)) for
X:(8192,256) f32, Y:(256,8192) f32, beta:(1,) f32 -> Z:(8192,8192) f32.

Sharding: X row-wise across 8 cores (1024 rows each); Y and beta replicated.
Each core computes a (1024, 8192) slab of Z; the host concatenates slabs.

Per-core algorithm:
  - cross term via PE matmul in bf16 with X scaled by -2 at convert time
    (exact power-of-two scaling), K=256 split into 2 chunks of 128.
  - ||y||^2 injected into the same PSUM accumulation as one extra fp16
    contraction row (ones (x) y2_fp16); fp16 keeps y2's absolute error
    ~0.12 on values ~512 (vs ~1-2 for bf16).
  - ||x||^2 (exact fp32) added via the per-partition bias of the fused
    ScalarE Sqrt activation reading PSUM.
  - z = beta - d in a single VectorE tensor_scalar pass: (d * -1) + beta.
"""

from contextlib import ExitStack

import numpy as np

import concourse.bacc as bacc
import concourse.mybir as mybir
import concourse.tile as tile
from concourse.bass_utils import run_bass_kernel_spmd
from concourse.masks import make_identity

N_CORES = 8
N_ROW, RANK, N_COL = 8192, 256, 8192
ROWS_PER_CORE = N_ROW // N_CORES  # 1024

P = 128      # partitions
FN = 512     # matmul free dim / PSUM bank (fp32)

f32 = mybir.dt.float32
bf16 = mybir.dt.bfloat16
f16 = mybir.dt.float16

AF = mybir.ActivationFunctionType
ALU = mybir.AluOpType


def build_l2_kernel(rows=ROWS_PER_CORE, rank=RANK, ncol=N_COL, n_cores=N_CORES,
                    out_bufs=8, psum_bufs=6,
                    use_e_row=True, use_beta_ap=True, use_x_side=True,
                    use_y_side=True, use_main=True,
                    use_x2=True, use_xT=True):
    """Build the per-core SPMD Bass program. Returns the compiled Bacc."""
    assert rows % P == 0 and rank % P == 0 and ncol % FN == 0
    mt = rows // P          # m-tiles (8)
    kc = rank // P          # k-chunks (2)
    nt = ncol // FN         # n-tiles (16)

    nc = bacc.Bacc("TRN2", target_bir_lowering=False, debug=False,
                   num_devices=n_cores)

    xs_d = nc.dram_tensor("Xs", [rows, rank], f32, kind="ExternalInput")
    y_d = nc.dram_tensor("Y", [rank, ncol], f32, kind="ExternalInput")
    beta_d = nc.dram_tensor("beta", [1, 1], f32, kind="ExternalInput")
    # Z stored as [mt, nt, 128, 512] tile blocks -> every DMA store is one
    # fully contiguous 256KB burst. Host reassembles to [rows, ncol].
    z_d = nc.dram_tensor("Z", [mt * nt * P, FN], f32, kind="ExternalOutput")

    with tile.TileContext(nc) as tc, ExitStack() as ctx:
        cpool = ctx.enter_context(tc.tile_pool(name="const", bufs=1))
        ypool = ctx.enter_context(tc.tile_pool(name="ybig", bufs=1))
        yf_pool = ctx.enter_context(tc.tile_pool(name="yf", bufs=6))
        ysq_pool = ctx.enter_context(tc.tile_pool(name="ysq", bufs=6))
        setup_psum = ExitStack()
        tp_psum = setup_psum.enter_context(
            tc.tile_pool(name="tpp", bufs=2, space="PSUM"))
        y2_psum = setup_psum.enter_context(
            tc.tile_pool(name="y2p", bufs=2, space="PSUM"))
        dpool = ctx.enter_context(tc.tile_pool(name="d", bufs=out_bufs))

        # ---- constants ----
        identity = cpool.tile([P, P], f32)
        make_identity(nc, identity[:])
        ones_row = cpool.tile([1, P], f16)       # lhsT of the y2-row matmul
        nc.gpsimd.memset(ones_row[:], 1.0)
        ones_col = cpool.tile([P, 1], bf16)      # lhsT of the y2 column-reduce
        nc.gpsimd.memset(ones_col[:], 1.0)
        if use_beta_ap:
            beta_b = cpool.tile([P, 1], f32)
            b11 = cpool.tile([1, 1], f32)
            nc.sync.dma_start(b11[:], beta_d.ap()[:])
            nc.gpsimd.partition_broadcast(beta_b[:], b11[:])
        else:
            beta_b = None

        # ---- X side: load slab, x2, transposed -2X in bf16 ----
        xs_sb = cpool.tile([P, mt, rank], f32)
        nc.sync.dma_start(
            xs_sb[:], xs_d.ap().rearrange("(t p) k -> p t k", p=P))

        x2 = cpool.tile([P, mt], f32)
        xsq = cpool.tile([P, rank], f32)
        xbT = [cpool.tile([P, rows], bf16, name=f"xbT{c}", tag=f"xbT{c}")
               for c in range(kc)]
        for m in range(mt if use_x_side else 0):
            if use_x2:
                nc.vector.tensor_tensor(
                    xsq[:], xs_sb[:, m, :], xs_sb[:, m, :], op=ALU.mult)
                nc.vector.reduce_sum(
                    x2[:, m : m + 1], xsq[:], axis=mybir.AxisListType.X)
            for c in range(kc if use_xT else 0):
                pt = tp_psum.tile([P, P], f32)
                nc.tensor.transpose(
                    pt[:], xs_sb[:, m, c * P : (c + 1) * P], identity[:])
                nc.scalar.activation(
                    xbT[c][:, m * P : (m + 1) * P], pt[:],
                    AF.Copy, scale=-2.0)

        # ---- Y side: stream pieces, convert to bf16, y2 -> fp16 row ----
        yb = [ypool.tile([P, ncol], bf16, name=f"yb{c}", tag=f"yb{c}")
              for c in range(kc)]
        e_row = cpool.tile([1, ncol], f16)
        for j in range(nt if use_y_side else 0):
            y2ps = y2_psum.tile([1, FN], f32)
            for c in range(kc):
                yf = yf_pool.tile([P, FN], f32)
                nc.sync.dma_start(
                    yf[:], y_d.ap()[c * P : (c + 1) * P,
                                    j * FN : (j + 1) * FN])
                nc.vector.tensor_copy(yb[c][:, j * FN : (j + 1) * FN], yf[:])
                ysq = ysq_pool.tile([P, FN], bf16)
                nc.scalar.activation(ysq[:], yf[:], AF.Square)
                nc.tensor.matmul(
                    y2ps[:], ones_col[:], ysq[:],
                    start=(c == 0), stop=(c == kc - 1))
            if use_e_row:
                nc.scalar.activation(
                    e_row[:, j * FN : (j + 1) * FN], y2ps[:], AF.Copy)

        # ---- main loop ----
        # setup PSUM pools released here -> main matmuls get 6 banks
        setup_psum.close()
        mm_psum = ctx.enter_context(
            tc.tile_pool(name="mmp", bufs=psum_bufs, space="PSUM"))
        for m in range(mt if (use_main and use_x_side and use_y_side) else 0):
            for j in range(nt):
                ps = mm_psum.tile([P, FN], f32)
                for c in range(kc):
                    nc.tensor.matmul(
                        ps[:], xbT[c][:, m * P : (m + 1) * P],
                        yb[c][:, j * FN : (j + 1) * FN],
                        start=(c == 0), stop=(not use_e_row and c == kc - 1))
                if use_e_row:
                    nc.tensor.matmul(
                        ps[:], ones_row[:],
                        e_row[:, j * FN : (j + 1) * FN],
                        start=False, stop=True)
                d = dpool.tile([P, FN], f32)
                nc.scalar.activation(
                    d[:], ps[:], AF.Sqrt, bias=x2[:, m : m + 1])
                nc.vector.tensor_scalar(
                    d[:], d[:], -1.0,
                    beta_b[:] if use_beta_ap else 0.0, ALU.mult, ALU.add)
                blk = (m * nt + j) * P
                nc.sync.dma_start(z_d.ap()[blk : blk + P, :], d[:])

    nc.compile()
    return nc


_CACHED = {}


def _get_nc():
    if "nc" not in _CACHED:
        _CACHED["nc"] = build_l2_kernel()
    return _CACHED["nc"]


def kernel(X, Y, beta):
    X = np.ascontiguousarray(np.asarray(X, dtype=np.float32))
    Y = np.ascontiguousarray(np.asarray(Y, dtype=np.float32))
    beta = np.asarray(beta, dtype=np.float32).reshape(1, 1)
    assert X.shape == (N_ROW, RANK) and Y.shape == (RANK, N_COL)

    nc = _get_nc()
    in_maps = [
        {"Xs": X[c * ROWS_PER_CORE : (c + 1) * ROWS_PER_CORE], "Y": Y,
         "beta": beta}
        for c in range(N_CORES)
    ]
    res = run_bass_kernel_spmd(nc, in_maps, core_ids=list(range(N_CORES)))
    mt, nt = ROWS_PER_CORE // P, N_COL // FN
    slabs = [
        res.results[c]["Z"].reshape(mt, nt, P, FN)
        .transpose(0, 2, 1, 3).reshape(ROWS_PER_CORE, N_COL)
        for c in range(N_CORES)
    ]
    return np.ascontiguousarray(np.concatenate(slabs, axis=0))

